# revision 5
# baseline (speedup 1.0000x reference)
# Dynamic sparse attention (sliding-window, paged-KV) on 8 TRN2 NeuronCores.
#
# Reference computation (B=2, S=2048, D=1024, H=16, HD=64, window=512):
#   q/k/v = x @ W{q,k,v}.T ; k/v scattered to a paged cache via slot_mapping,
#   gathered back via block_tables ; causal sliding-window attention ;
#   out = attn @ wo.T
#
# Sharding: core c in 0..7 -> batch bi=c//4, head-group hg=c%4 (4 heads each).
# Each core reads only its batch's activations (pre-transposed + bf16-cast on
# host) and its head-group's weight slices, and writes a partial output
# transpose outT [D, S] (f32). Host sums the 4 head-group partials per batch
# and transposes back. The paged-cache scatter/gather composes to a single
# token-gather g (identity for the arange block_tables/slot_mapping); it is
# folded into a host-side column gather of x for the K/V projection input.
#
# On-device layout (per core):
#   qT/kT  [128, 2, 2048] bf16   (partition = head-dim pair, free = seq)
#   V^     [128, 16*260] bf16    (keys on partitions; per head 64 V cols +
#                                 a ones column -> PV matmul also accumulates
#                                 the softmax denominator Z for free)
#   scores are computed transposed (S^T[k, q]) per 128-key strip so the
#   exp'd strip feeds the PV matmul directly as the moving operand -- no
#   P transposes. No running-max is needed (scores ~ N(0,1) after 1/8 scale);
#   masked entries are zeroed post-exp by a 0/1 mask multiply on DVE.

import numpy as np

import concourse.bass as bass
import concourse.tile as tile
from concourse import bacc, mybir
from concourse.bass_utils import run_bass_kernel_spmd

B, S, D, H, HD = 2, 2048, 1024, 16, 64
BLOCK = 16
WINDOW = 512
P = 128
NCORES = 8
HPC = 4          # heads per core
CW = HPC * HD    # per-core projection width = 256
NKB = S // P     # 16 key blocks
NQT = S // 512   # 4 q-tiles of 512
FP32 = mybir.dt.float32
BF16 = mybir.dt.bfloat16
VROW = HPC * 65  # vhat cols per key block (4 heads x (64 V + 1 ones))


def _strip_width(kb: int) -> int:
    return min(512 + P, S - P * kb)


def _emit(ctx, nc, tc, xT, xTg, wqkvT, woT, mask2, ident, outT):
    const = ctx.enter_context(tc.tile_pool(name="const", bufs=1))
    xs_pool = ctx.enter_context(tc.tile_pool(name="xs", bufs=2))
    acts = ctx.enter_context(tc.tile_pool(name="acts", bufs=1))
    vt_pool = ctx.enter_context(tc.tile_pool(name="vt", bufs=3))
    strip_pool = ctx.enter_context(tc.tile_pool(name="strips", bufs=9))
    z_pool = ctx.enter_context(tc.tile_pool(name="zch", bufs=4))
    out_pool = ctx.enter_context(tc.tile_pool(name="wo_out", bufs=4))
    psum_mm = ctx.enter_context(tc.tile_pool(name="mm512", bufs=2, space="PSUM"))
    psum_sc = ctx.enter_context(tc.tile_pool(name="pscore", bufs=2, space="PSUM"))
    psum_pv = ctx.enter_context(tc.tile_pool(name="ppv", bufs=2, space="PSUM"))

    # ---- constants ----
    wqkv_s = const.tile([P, 8 * 3 * CW], BF16, name="wqkv_s")
    nc.sync.dma_start(
        out=wqkv_s[:].rearrange("p (dt c) -> p dt c", dt=8),
        in_=wqkvT.rearrange("(dt p) c -> p dt c", p=P),
    )
    woT_s = const.tile([P, 2 * D], BF16, name="woT_s")
    nc.sync.dma_start(
        out=woT_s[:].rearrange("p (jt o) -> p jt o", jt=2),
        in_=woT.rearrange("(jt p) o -> p jt o", p=P),
    )
    mask_s = const.tile([P, 2 * P], BF16, name="mask_s")
    nc.sync.dma_start(out=mask_s[:], in_=mask2[:, :])
    ident_s = const.tile([P, P], BF16, name="ident_s")
    nc.sync.dma_start(out=ident_s[:], in_=ident[:, :])

    # ---- persistent activations ----
    qT_s = acts.tile([P, 2 * S], BF16, name="qT_s")
    kT_s = acts.tile([P, 2 * S], BF16, name="kT_s")
    vhat = acts.tile([P, NKB * VROW], BF16, name="vhat")
    attnT = acts.tile([P, 2 * S], BF16, name="attnT")
    for h in range(HPC):
        nc.vector.memset(
            vhat[:].rearrange("p (kb c) -> p kb c", kb=NKB)[:, :, 65 * h + 64 : 65 * h + 65],
            1.0,
        )

    # ---- phase 1: projections  qT/kT/vT = W^T.T @ x^T ----
    with nc.named_scope("proj"):
        for sst in range(2):  # 1024-wide supertiles of seq
            c0 = 1024 * sst
            xs = xs_pool.tile([P, 8 * 1024], BF16, tag="xs", name="xs")
            nc.sync.dma_start(
                out=xs[:].rearrange("p (dt s) -> p dt s", dt=8),
                in_=xT[:, c0 : c0 + 1024].rearrange("(dt p) s -> p dt s", p=P),
            )
            xg = xs_pool.tile([P, 8 * 1024], BF16, tag="xg", name="xg")
            nc.sync.dma_start(
                out=xg[:].rearrange("p (dt s) -> p dt s", dt=8),
                in_=xTg[:, c0 : c0 + 1024].rearrange("(dt p) s -> p dt s", p=P),
            )
            for half in range(2):  # N=512 matmul tiles
                scol = c0 + 512 * half
                for proj in range(3):  # q, k, v
                    src = xs if proj == 0 else xg
                    for dto in range(2):
                        ps = psum_mm.tile([P, 512], FP32, tag="mm512", name="ps_proj")
                        for dt in range(8):
                            nc.tensor.matmul(
                                ps[:],
                                wqkv_s[:, dt * 3 * CW + CW * proj + P * dto : dt * 3 * CW + CW * proj + P * dto + P],
                                src[:, dt * 1024 + 512 * half : dt * 1024 + 512 * half + 512],
                                start=(dt == 0),
                                stop=(dt == 7),
                            )
                        if proj == 0:
                            nc.any.tensor_copy(out=qT_s[:, S * dto + scol : S * dto + scol + 512], in_=ps[:])
                        elif proj == 1:
                            nc.any.tensor_copy(out=kT_s[:, S * dto + scol : S * dto + scol + 512], in_=ps[:])
                        else:
                            # v: evict to bf16, transpose 128x128 chunks into vhat
                            vt = vt_pool.tile([P, 512], BF16, tag="vt", name="vt")
                            nc.any.tensor_copy(out=vt[:], in_=ps[:])
                            for j in range(4):
                                kb = scol // P + j
                                tr = psum_sc.tile([P, P], BF16, tag="score", name="tr")
                                nc.tensor.transpose(tr[:], vt[:, P * j : P * j + P], ident_s[:])
                                for hh in range(2):  # heads 2*dto, 2*dto+1
                                    h = 2 * dto + hh
                                    nc.any.tensor_copy(
                                        out=vhat[:, VROW * kb + 65 * h : VROW * kb + 65 * h + 64],
                                        in_=tr[:, 64 * hh : 64 * hh + 64],
                                    )

    # ---- phase 2: attention per head ----
    for h in range(HPC):
        ht, hp = h // 2, 64 * (h % 2)
        with nc.named_scope(f"attn_h{h}"):
            strips = {}
            for kb in range(NKB):
                w = _strip_width(kb)
                n1 = min(512, w)
                n2 = w - n1
                ps = psum_sc.tile([P, w], FP32, tag="score", name="ps_sc")
                lhsT = kT_s[hp : hp + 64, S * ht + P * kb : S * ht + P * kb + P]
                nc.tensor.matmul(
                    ps[:, 0:n1],
                    lhsT,
                    qT_s[hp : hp + 64, S * ht + P * kb : S * ht + P * kb + n1],
                    start=True,
                    stop=True,
                )
                if n2:
                    nc.tensor.matmul(
                        ps[:, 512 : 512 + n2],
                        lhsT,
                        qT_s[hp : hp + 64, S * ht + P * kb + 512 : S * ht + P * kb + 512 + n2],
                        start=True,
                        stop=True,
                    )
                st = strip_pool.tile([P, w], BF16, tag="strip", name="strip")
                nc.scalar.activation(st[:], ps[:], mybir.ActivationFunctionType.Exp, scale=float(HD) ** -0.5)
                # zero the invalid triangles: diag block (cols 0:128) and tail block
                nc.vector.tensor_mul(out=st[:, 0:P], in0=st[:, 0:P], in1=mask_s[:, 0:P])
                if n2:
                    nc.vector.tensor_mul(
                        out=st[:, 512 : 512 + n2], in0=st[:, 512 : 512 + n2], in1=mask_s[:, P : P + n2]
                    )
                strips[kb] = st

                # PV for q-tile qt once its last strip (kb == 4qt+3 or last) is done
                if kb >= 3 and (kb - 3) % 4 == 0:
                    qt = (kb - 3) // 4
                    pv = psum_pv.tile([65, 512], FP32, tag="pv", name="ps_pv")
                    for qb in range(4 * qt, 4 * qt + 4):
                        for kb2 in range(max(0, qb - 4), qb + 1):
                            nc.tensor.matmul(
                                pv[:, P * (qb - 4 * qt) : P * (qb - 4 * qt) + P],
                                vhat[:, VROW * kb2 + 65 * h : VROW * kb2 + 65 * h + 65],
                                strips[kb2][:, P * (qb - kb2) : P * (qb - kb2) + P],
                                start=(kb2 == max(0, qb - 4)),
                                stop=(kb2 == qb),
                            )
                    zr = z_pool.tile([1, 512], FP32, tag="zr", name="zr")
                    nc.vector.reciprocal(zr[:], pv[64:65, :])
                    zrb = z_pool.tile([64, 512], FP32, tag="zrb", name="zrb")
                    nc.gpsimd.partition_broadcast(zrb[:], zr[:])
                    nc.vector.tensor_mul(
                        out=attnT[hp : hp + 64, S * ht + 512 * qt : S * ht + 512 * qt + 512],
                        in0=pv[0:64, :],
                        in1=zrb[:],
                    )

    # ---- phase 3: output projection (partial over this core's heads) ----
    with nc.named_scope("wo"):
        for st_i in range(NQT):
            s0 = 512 * st_i
            for ot in range(8):
                ps = psum_mm.tile([P, 512], FP32, tag="mm512", name="ps_wo")
                for jt in range(2):
                    nc.tensor.matmul(
                        ps[:],
                        woT_s[:, D * jt + P * ot : D * jt + P * ot + P],
                        attnT[:, S * jt + s0 : S * jt + s0 + 512],
                        start=(jt == 0),
                        stop=(jt == 1),
                    )
                ob = out_pool.tile([P, 512], FP32, tag="wo", name="ob")
                nc.any.tensor_copy(out=ob[:], in_=ps[:])
                nc.sync.dma_start(out=outT[P * ot : P * ot + P, s0 : s0 + 512], in_=ob[:])


_GRAPH_CACHE = {}


def _build():
    if "nc" in _GRAPH_CACHE:
        return _GRAPH_CACHE["nc"]
    nc = bacc.Bacc("TRN2", target_bir_lowering=False, debug=False, num_devices=NCORES)
    xT = nc.dram_tensor("xT", [D, S], BF16, kind="ExternalInput")
    xTg = nc.dram_tensor("xTg", [D, S], BF16, kind="ExternalInput")
    wqkvT = nc.dram_tensor("wqkvT", [D, 3 * CW], BF16, kind="ExternalInput")
    woT = nc.dram_tensor("woT", [CW, D], BF16, kind="ExternalInput")
    mask2 = nc.dram_tensor("mask2", [P, 2 * P], BF16, kind="ExternalInput")
    ident = nc.dram_tensor("ident", [P, P], BF16, kind="ExternalInput")
    outT = nc.dram_tensor("outT", [D, S], FP32, kind="ExternalOutput")
    from contextlib import ExitStack

    with tile.TileContext(nc) as tc, ExitStack() as ctx:
        _emit(ctx, nc, tc, xT, xTg, wqkvT, woT, mask2, ident, outT)
    nc.compile()
    _GRAPH_CACHE["nc"] = nc
    return nc


def _host_masks():
    p = np.arange(P)[:, None]
    c = np.arange(P)[None, :]
    diag = (p <= c).astype(np.float32)   # causal within the diagonal block
    tail = (p > c).astype(np.float32)    # q-k <= 511 within the tail block
    return np.concatenate([diag, tail], axis=1)


def _token_gather(block_tables, slot_mapping):
    """Compose cache scatter (slot_mapping) with block_tables gather into a
    single token index map g[b, t] -> row of x_flat."""
    t = np.arange(S)
    slots = block_tables[:, t // BLOCK].astype(np.int64) * BLOCK + (t % BLOCK)[None, :]
    sm = np.asarray(slot_mapping).astype(np.int64)
    sm_inv = np.empty_like(sm)
    sm_inv[sm] = np.arange(sm.size)
    return sm_inv[slots]  # [B, S]


def make_in_maps(x, wq, wk, wv, wo, block_tables, slot_mapping):
    bf = mybir.dt.np(BF16)
    g = _token_gather(np.asarray(block_tables), np.asarray(slot_mapping))
    x_flat = np.ascontiguousarray(np.asarray(x, dtype=np.float32).reshape(B * S, D))
    mask2 = _host_masks().astype(bf)
    ident = np.eye(P, dtype=np.float32).astype(bf)
    wq, wk, wv, wo = (np.asarray(a, dtype=np.float32) for a in (wq, wk, wv, wo))

    xT_b, xTg_b = [], []
    for bi in range(B):
        xT_b.append(np.ascontiguousarray(x_flat[bi * S : (bi + 1) * S].T.astype(bf)))
        gb = g[bi]
        if np.array_equal(gb, np.arange(bi * S, (bi + 1) * S)):
            xTg_b.append(xT_b[-1])
        else:
            xTg_b.append(np.ascontiguousarray(x_flat[gb].T.astype(bf)))

    in_maps = []
    for c in range(NCORES):
        bi, hg = c // 4, c % 4
        rows = slice(CW * hg, CW * hg + CW)
        wqkvT = np.ascontiguousarray(
            np.concatenate([wq[rows].T, wk[rows].T, wv[rows].T], axis=1).astype(bf)
        )
        woT = np.ascontiguousarray(wo[:, rows].T.astype(bf))
        in_maps.append(
            {
                "xT": xT_b[bi],
                "xTg": xTg_b[bi],
                "wqkvT": wqkvT,
                "woT": woT,
                "mask2": mask2,
                "ident": ident,
            }
        )
    return in_maps


def kernel(x, wq, wk, wv, wo, block_tables, slot_mapping, context_lens, window_size, **run_kwargs):
    assert int(window_size) == WINDOW, f"kernel hardcodes window {WINDOW}"
    assert tuple(np.asarray(x).shape) == (B, S, D)
    nc = _build()
    in_maps = make_in_maps(x, wq, wk, wv, wo, block_tables, slot_mapping)
    res = run_bass_kernel_spmd(nc, in_maps, core_ids=list(range(NCORES)), **run_kwargs)
    outs = [r["outT"].astype(np.float32) for r in res.results]
    out = np.stack(
        [sum(outs[4 * bi : 4 * bi + 4]).T for bi in range(B)]
    ).reshape(B, S, D)
    # context_lens == S for these inputs (full visibility); asserted cheaply
    assert np.all(np.asarray(context_lens) == S)
    if run_kwargs:
        kernel.last_result = res
    return out


# revision 9
# speedup vs baseline: 1.0370x; 1.0370x over previous
# Dynamic sparse attention (sliding-window, paged-KV) on 8 TRN2 NeuronCores.
#
# Reference computation (B=2, S=2048, D=1024, H=16, HD=64, window=512):
#   q/k/v = x @ W{q,k,v}.T ; k/v scattered to a paged cache via slot_mapping,
#   gathered back via block_tables ; causal sliding-window attention ;
#   out = attn @ wo.T
#
# Sharding: core c in 0..7 -> batch bi=c//4, head-group hg=c%4 (4 heads each).
# Each core reads only its batch's activations (pre-transposed + bf16-cast on
# host) and its head-group's weight slices, and writes a partial output
# transpose outT [D, S] (f32). Host sums the 4 head-group partials per batch
# and transposes back. The paged-cache scatter/gather composes to a single
# token-gather g (identity for the arange block_tables/slot_mapping); it is
# folded into a host-side column gather of x for the K/V projection input.
#
# On-device layout (per core):
#   qT/kT  [128, 2, 2048] bf16   (partition = head-dim pair, free = seq)
#   V^     [128, 16*260] bf16    (keys on partitions; per head 64 V cols +
#                                 a ones column -> PV matmul also accumulates
#                                 the softmax denominator Z for free)
#   scores are computed transposed (S^T[k, q]) per 128-key strip so the
#   exp'd strip feeds the PV matmul directly as the moving operand -- no
#   P transposes. No running-max is needed (scores ~ N(0,1) after 1/8 scale);
#   masked entries are zeroed post-exp by a 0/1 mask multiply on DVE.

import numpy as np

import concourse.bass as bass
import concourse.tile as tile
from concourse import bacc, mybir
from concourse.bass_utils import run_bass_kernel_spmd

B, S, D, H, HD = 2, 2048, 1024, 16, 64
BLOCK = 16
WINDOW = 512
P = 128
NCORES = 8
HPC = 4          # heads per core
CW = HPC * HD    # per-core projection width = 256
NKB = S // P     # 16 key blocks
NQT = S // 512   # 4 q-tiles of 512
FP32 = mybir.dt.float32
BF16 = mybir.dt.bfloat16
VROW = HPC * 65  # vhat cols per key block (4 heads x (64 V + 1 ones))


def _strip_width(kb: int) -> int:
    return min(512 + P, S - P * kb)


def _emit(ctx, nc, tc, xT, xTg, wqkvT, woT, mask2, ident, outT):
    const = ctx.enter_context(tc.tile_pool(name="const", bufs=1))
    xs_pool = ctx.enter_context(tc.tile_pool(name="xs", bufs=2))
    acts = ctx.enter_context(tc.tile_pool(name="acts", bufs=1))
    vt_pool = ctx.enter_context(tc.tile_pool(name="vt", bufs=3))
    strip_pool = ctx.enter_context(tc.tile_pool(name="strips", bufs=34))
    z_pool = ctx.enter_context(tc.tile_pool(name="zch", bufs=4))
    out_pool = ctx.enter_context(tc.tile_pool(name="wo_out", bufs=4))
    psum_mm = ctx.enter_context(tc.tile_pool(name="mm512", bufs=2, space="PSUM"))
    psum_sc = ctx.enter_context(tc.tile_pool(name="pscore", bufs=2, space="PSUM"))
    psum_pv = ctx.enter_context(tc.tile_pool(name="ppv", bufs=2, space="PSUM"))

    # ---- constants ----
    wqkv_s = const.tile([P, 8 * 3 * CW], BF16, name="wqkv_s")
    nc.sync.dma_start(
        out=wqkv_s[:].rearrange("p (dt c) -> p dt c", dt=8),
        in_=wqkvT.rearrange("(dt p) c -> p dt c", p=P),
    )
    woT_s = const.tile([P, 2 * D], BF16, name="woT_s")
    nc.sync.dma_start(
        out=woT_s[:].rearrange("p (jt o) -> p jt o", jt=2),
        in_=woT.rearrange("(jt p) o -> p jt o", p=P),
    )
    mask_s = const.tile([P, 2 * P], BF16, name="mask_s")
    nc.sync.dma_start(out=mask_s[:], in_=mask2[:, :])
    ident_s = const.tile([P, P], BF16, name="ident_s")
    nc.sync.dma_start(out=ident_s[:], in_=ident[:, :])

    # ---- persistent activations ----
    qT_s = acts.tile([P, 2 * S], BF16, name="qT_s")
    kT_s = acts.tile([P, 2 * S], BF16, name="kT_s")
    vhat = acts.tile([P, NKB * VROW], BF16, name="vhat")
    # per-qt attnT tiles so the wo stage for q-tile qt only depends on that
    # qt's PV evictions (lets wo overlap the next qt's strips)
    attnT_q = [acts.tile([P, 2 * 512], BF16, name=f"attnT{i}") for i in range(NQT)]
    for h in range(HPC):
        nc.vector.memset(
            vhat[:].rearrange("p (kb c) -> p kb c", kb=NKB)[:, :, 65 * h + 64 : 65 * h + 65],
            1.0,
        )

    # ---- phase 1: projections  qT/kT/vT = W^T.T @ x^T ----
    with nc.named_scope("proj"):
        for sst in range(2):  # 1024-wide supertiles of seq
            c0 = 1024 * sst
            xs = xs_pool.tile([P, 8 * 1024], BF16, tag="xs", name="xs")
            nc.sync.dma_start(
                out=xs[:].rearrange("p (dt s) -> p dt s", dt=8),
                in_=xT[:, c0 : c0 + 1024].rearrange("(dt p) s -> p dt s", p=P),
            )
            xg = xs_pool.tile([P, 8 * 1024], BF16, tag="xg", name="xg")
            nc.sync.dma_start(
                out=xg[:].rearrange("p (dt s) -> p dt s", dt=8),
                in_=xTg[:, c0 : c0 + 1024].rearrange("(dt p) s -> p dt s", p=P),
            )
            for half in range(2):  # N=512 matmul tiles
                scol = c0 + 512 * half
                for proj in range(3):  # q, k, v
                    src = xs if proj == 0 else xg
                    for dto in range(2):
                        ps = psum_mm.tile([P, 512], FP32, tag="mm512", name="ps_proj")
                        for dt in range(8):
                            nc.tensor.matmul(
                                ps[:],
                                wqkv_s[:, dt * 3 * CW + CW * proj + P * dto : dt * 3 * CW + CW * proj + P * dto + P],
                                src[:, dt * 1024 + 512 * half : dt * 1024 + 512 * half + 512],
                                start=(dt == 0),
                                stop=(dt == 7),
                            )
                        if proj == 0:
                            nc.any.tensor_copy(out=qT_s[:, S * dto + scol : S * dto + scol + 512], in_=ps[:])
                        elif proj == 1:
                            nc.any.tensor_copy(out=kT_s[:, S * dto + scol : S * dto + scol + 512], in_=ps[:])
                        else:
                            # v: evict to bf16, transpose 128x128 chunks into vhat
                            vt = vt_pool.tile([P, 512], BF16, tag="vt", name="vt")
                            nc.any.tensor_copy(out=vt[:], in_=ps[:])
                            for j in range(4):
                                kb = scol // P + j
                                tr = psum_sc.tile([P, P], BF16, tag="score", name="tr")
                                nc.tensor.transpose(tr[:], vt[:, P * j : P * j + P], ident_s[:])
                                for hh in range(2):  # heads 2*dto, 2*dto+1
                                    h = 2 * dto + hh
                                    nc.any.tensor_copy(
                                        out=vhat[:, VROW * kb + 65 * h : VROW * kb + 65 * h + 64],
                                        in_=tr[:, 64 * hh : 64 * hh + 64],
                                    )

    # ---- phase 2+3: qt-major pipeline. Strips of the 4 heads are emitted
    # interleaved (independent chains keep the PE dense -> HAM stays at full
    # clock); each qt's wo matmuls run as soon as its attnT is normalized,
    # overlapping the next qt's strips.
    strips = {h: {} for h in range(HPC)}
    for qt in range(NQT):
        with nc.named_scope(f"attn_q{qt}"):
            for kb in range(4 * qt, 4 * qt + 4):
                for h in range(HPC):
                    ht, hp = h // 2, 64 * (h % 2)
                    w = _strip_width(kb)
                    n1 = min(512, w)
                    n2 = w - n1
                    ps = psum_sc.tile([P, w], FP32, tag="score", name="ps_sc")
                    lhsT = kT_s[hp : hp + 64, S * ht + P * kb : S * ht + P * kb + P]
                    nc.tensor.matmul(
                        ps[:, 0:n1],
                        lhsT,
                        qT_s[hp : hp + 64, S * ht + P * kb : S * ht + P * kb + n1],
                        start=True,
                        stop=True,
                    )
                    if n2:
                        nc.tensor.matmul(
                            ps[:, 512 : 512 + n2],
                            lhsT,
                            qT_s[hp : hp + 64, S * ht + P * kb + 512 : S * ht + P * kb + 512 + n2],
                            start=True,
                            stop=True,
                        )
                    st = strip_pool.tile([P, w], BF16, tag="strip", name="strip")
                    nc.scalar.activation(
                        st[:], ps[:], mybir.ActivationFunctionType.Exp, scale=float(HD) ** -0.5
                    )
                    # zero invalid triangles: diag block (cols 0:128), tail block
                    nc.vector.tensor_mul(out=st[:, 0:P], in0=st[:, 0:P], in1=mask_s[:, 0:P])
                    if n2:
                        nc.vector.tensor_mul(
                            out=st[:, 512 : 512 + n2],
                            in0=st[:, 512 : 512 + n2],
                            in1=mask_s[:, P : P + n2],
                        )
                    strips[h][kb] = st

            for h in range(HPC):
                ht, hp = h // 2, 64 * (h % 2)
                pv = psum_pv.tile([65, 512], FP32, tag="pv", name="ps_pv")
                for qb in range(4 * qt, 4 * qt + 4):
                    for kb2 in range(max(0, qb - 4), qb + 1):
                        nc.tensor.matmul(
                            pv[:, P * (qb - 4 * qt) : P * (qb - 4 * qt) + P],
                            vhat[:, VROW * kb2 + 65 * h : VROW * kb2 + 65 * h + 65],
                            strips[h][kb2][:, P * (qb - kb2) : P * (qb - kb2) + P],
                            start=(kb2 == max(0, qb - 4)),
                            stop=(kb2 == qb),
                        )
                zr = z_pool.tile([1, 512], FP32, tag="zr", name="zr")
                nc.vector.reciprocal(zr[:], pv[64:65, :])
                zrb = z_pool.tile([64, 512], FP32, tag="zrb", name="zrb")
                nc.gpsimd.partition_broadcast(zrb[:], zr[:])
                nc.vector.tensor_mul(
                    out=attnT_q[qt][hp : hp + 64, 512 * ht : 512 * ht + 512],
                    in0=pv[0:64, :],
                    in1=zrb[:],
                )

        with nc.named_scope(f"wo_q{qt}"):
            for ot in range(8):
                ps = psum_mm.tile([P, 512], FP32, tag="mm512", name="ps_wo")
                for jt in range(2):
                    nc.tensor.matmul(
                        ps[:],
                        woT_s[:, D * jt + P * ot : D * jt + P * ot + P],
                        attnT_q[qt][:, 512 * jt : 512 * jt + 512],
                        start=(jt == 0),
                        stop=(jt == 1),
                    )
                ob = out_pool.tile([P, 512], FP32, tag="wo", name="ob")
                nc.any.tensor_copy(out=ob[:], in_=ps[:])
                nc.sync.dma_start(out=outT[P * ot : P * ot + P, 512 * qt : 512 * qt + 512], in_=ob[:])


_GRAPH_CACHE = {}


def _build():
    if "nc" in _GRAPH_CACHE:
        return _GRAPH_CACHE["nc"]
    nc = bacc.Bacc("TRN2", target_bir_lowering=False, debug=False, num_devices=NCORES)
    xT = nc.dram_tensor("xT", [D, S], BF16, kind="ExternalInput")
    xTg = nc.dram_tensor("xTg", [D, S], BF16, kind="ExternalInput")
    wqkvT = nc.dram_tensor("wqkvT", [D, 3 * CW], BF16, kind="ExternalInput")
    woT = nc.dram_tensor("woT", [CW, D], BF16, kind="ExternalInput")
    mask2 = nc.dram_tensor("mask2", [P, 2 * P], BF16, kind="ExternalInput")
    ident = nc.dram_tensor("ident", [P, P], BF16, kind="ExternalInput")
    outT = nc.dram_tensor("outT", [D, S], FP32, kind="ExternalOutput")
    from contextlib import ExitStack

    with tile.TileContext(nc) as tc, ExitStack() as ctx:
        _emit(ctx, nc, tc, xT, xTg, wqkvT, woT, mask2, ident, outT)
    nc.compile()
    _GRAPH_CACHE["nc"] = nc
    return nc


def _host_masks():
    p = np.arange(P)[:, None]
    c = np.arange(P)[None, :]
    diag = (p <= c).astype(np.float32)   # causal within the diagonal block
    tail = (p > c).astype(np.float32)    # q-k <= 511 within the tail block
    return np.concatenate([diag, tail], axis=1)


def _token_gather(block_tables, slot_mapping):
    """Compose cache scatter (slot_mapping) with block_tables gather into a
    single token index map g[b, t] -> row of x_flat."""
    t = np.arange(S)
    slots = block_tables[:, t // BLOCK].astype(np.int64) * BLOCK + (t % BLOCK)[None, :]
    sm = np.asarray(slot_mapping).astype(np.int64)
    sm_inv = np.empty_like(sm)
    sm_inv[sm] = np.arange(sm.size)
    return sm_inv[slots]  # [B, S]


def make_in_maps(x, wq, wk, wv, wo, block_tables, slot_mapping):
    bf = mybir.dt.np(BF16)
    g = _token_gather(np.asarray(block_tables), np.asarray(slot_mapping))
    x_flat = np.ascontiguousarray(np.asarray(x, dtype=np.float32).reshape(B * S, D))
    mask2 = _host_masks().astype(bf)
    ident = np.eye(P, dtype=np.float32).astype(bf)
    wq, wk, wv, wo = (np.asarray(a, dtype=np.float32) for a in (wq, wk, wv, wo))

    xT_b, xTg_b = [], []
    for bi in range(B):
        xT_b.append(np.ascontiguousarray(x_flat[bi * S : (bi + 1) * S].T.astype(bf)))
        gb = g[bi]
        if np.array_equal(gb, np.arange(bi * S, (bi + 1) * S)):
            xTg_b.append(xT_b[-1])
        else:
            xTg_b.append(np.ascontiguousarray(x_flat[gb].T.astype(bf)))

    in_maps = []
    for c in range(NCORES):
        bi, hg = c // 4, c % 4
        rows = slice(CW * hg, CW * hg + CW)
        wqkvT = np.ascontiguousarray(
            np.concatenate([wq[rows].T, wk[rows].T, wv[rows].T], axis=1).astype(bf)
        )
        woT = np.ascontiguousarray(wo[:, rows].T.astype(bf))
        in_maps.append(
            {
                "xT": xT_b[bi],
                "xTg": xTg_b[bi],
                "wqkvT": wqkvT,
                "woT": woT,
                "mask2": mask2,
                "ident": ident,
            }
        )
    return in_maps


def kernel(x, wq, wk, wv, wo, block_tables, slot_mapping, context_lens, window_size, **run_kwargs):
    assert int(window_size) == WINDOW, f"kernel hardcodes window {WINDOW}"
    assert tuple(np.asarray(x).shape) == (B, S, D)
    nc = _build()
    in_maps = make_in_maps(x, wq, wk, wv, wo, block_tables, slot_mapping)
    res = run_bass_kernel_spmd(nc, in_maps, core_ids=list(range(NCORES)), **run_kwargs)
    outs = [r["outT"].astype(np.float32) for r in res.results]
    out = np.stack(
        [sum(outs[4 * bi : 4 * bi + 4]).T for bi in range(B)]
    ).reshape(B, S, D)
    # context_lens == S for these inputs (full visibility); asserted cheaply
    assert np.all(np.asarray(context_lens) == S)
    if run_kwargs:
        kernel.last_result = res
    return out


# revision 11
# speedup vs baseline: 1.3182x; 1.2711x over previous
# Dynamic sparse attention (sliding-window, paged-KV) on 8 TRN2 NeuronCores.
#
# Reference computation (B=2, S=2048, D=1024, H=16, HD=64, window=512):
#   q/k/v = x @ W{q,k,v}.T ; k/v scattered to a paged cache via slot_mapping,
#   gathered back via block_tables ; causal sliding-window attention ;
#   out = attn @ wo.T
#
# Sharding: core c in 0..7 -> batch bi=c//4, head-group hg=c%4 (4 heads each).
# Each core reads only its batch's activations (pre-transposed + bf16-cast on
# host) and its head-group's weight slices, and writes a partial output
# transpose outT [D, S] (f32). Host sums the 4 head-group partials per batch
# and transposes back. The paged-cache scatter/gather composes to a single
# token-gather g (identity for the arange block_tables/slot_mapping); it is
# folded into a host-side column gather of x for the K/V projection input.
#
# On-device layout (per core):
#   qT/kT  [128, 2, 2048] bf16   (partition = head-dim pair, free = seq)
#   V^     [128, 16*260] bf16    (keys on partitions; per head 64 V cols +
#                                 a ones column -> PV matmul also accumulates
#                                 the softmax denominator Z for free)
#   scores are computed transposed (S^T[k, q]) per 128-key strip so the
#   exp'd strip feeds the PV matmul directly as the moving operand -- no
#   P transposes. No running-max is needed (scores ~ N(0,1) after 1/8 scale);
#   masked entries are zeroed post-exp by a 0/1 mask multiply on DVE.

import numpy as np

import concourse.bass as bass
import concourse.tile as tile
from concourse import bacc, mybir
from concourse.bass_utils import run_bass_kernel_spmd

B, S, D, H, HD = 2, 2048, 1024, 16, 64
BLOCK = 16
WINDOW = 512
P = 128
NCORES = 8
HPC = 4          # heads per core
CW = HPC * HD    # per-core projection width = 256
NKB = S // P     # 16 key blocks
NQT = S // 512   # 4 q-tiles of 512
FP32 = mybir.dt.float32
BF16 = mybir.dt.bfloat16
VROW = HPC * 65  # vhat cols per key block (4 heads x (64 V + 1 ones))


def _strip_width(kb: int) -> int:
    return min(512 + P, S - P * kb)


def _emit(ctx, nc, tc, xT, xTg, wqkvT, woT, mask2, ident, outT):
    const = ctx.enter_context(tc.tile_pool(name="const", bufs=1))
    xs_pool = ctx.enter_context(tc.tile_pool(name="xs", bufs=2))
    acts = ctx.enter_context(tc.tile_pool(name="acts", bufs=1))
    vt_pool = ctx.enter_context(tc.tile_pool(name="vt", bufs=3))
    strip_pool = ctx.enter_context(tc.tile_pool(name="strips", bufs=34))
    z_pool = ctx.enter_context(tc.tile_pool(name="zch", bufs=4))
    out_pool = ctx.enter_context(tc.tile_pool(name="wo_out", bufs=4))
    psum_mm = ctx.enter_context(tc.tile_pool(name="mm512", bufs=2, space="PSUM"))
    psum_sc = ctx.enter_context(tc.tile_pool(name="pscore", bufs=2, space="PSUM"))
    psum_pv = ctx.enter_context(tc.tile_pool(name="ppv", bufs=2, space="PSUM"))

    # ---- constants ----
    wqkv_s = const.tile([P, 8 * 3 * CW], BF16, name="wqkv_s")
    nc.sync.dma_start(
        out=wqkv_s[:].rearrange("p (dt c) -> p dt c", dt=8),
        in_=wqkvT.rearrange("(dt p) c -> p dt c", p=P),
    )
    woT_s = const.tile([P, 2 * D], BF16, name="woT_s")
    nc.sync.dma_start(
        out=woT_s[:].rearrange("p (jt o) -> p jt o", jt=2),
        in_=woT.rearrange("(jt p) o -> p jt o", p=P),
    )
    mask_s = const.tile([P, 2 * P], BF16, name="mask_s")
    nc.sync.dma_start(out=mask_s[:], in_=mask2[:, :])
    ident_s = const.tile([P, P], BF16, name="ident_s")
    nc.sync.dma_start(out=ident_s[:], in_=ident[:, :])

    # ---- persistent activations ----
    qT_s = acts.tile([P, 2 * S], BF16, name="qT_s")
    kT_s = acts.tile([P, 2 * S], BF16, name="kT_s")
    vhat = acts.tile([P, NKB * VROW], BF16, name="vhat")
    # per-qt attnT tiles so the wo stage for q-tile qt only depends on that
    # qt's PV evictions (lets wo overlap the next qt's strips)
    attnT_q = [acts.tile([P, 2 * 512], BF16, name=f"attnT{i}") for i in range(NQT)]
    for h in range(HPC):
        nc.vector.memset(
            vhat[:].rearrange("p (kb c) -> p kb c", kb=NKB)[:, :, 65 * h + 64 : 65 * h + 65],
            1.0,
        )

    # ---- phase 1: projections  qT/kT/vT = W^T.T @ x^T ----
    with nc.named_scope("proj"):
        for sst in range(2):  # 1024-wide supertiles of seq
            c0 = 1024 * sst
            xs = xs_pool.tile([P, 8 * 1024], BF16, tag="xs", name="xs")
            nc.sync.dma_start(
                out=xs[:].rearrange("p (dt s) -> p dt s", dt=8),
                in_=xT[:, c0 : c0 + 1024].rearrange("(dt p) s -> p dt s", p=P),
            )
            xg = xs_pool.tile([P, 8 * 1024], BF16, tag="xg", name="xg")
            nc.sync.dma_start(
                out=xg[:].rearrange("p (dt s) -> p dt s", dt=8),
                in_=xTg[:, c0 : c0 + 1024].rearrange("(dt p) s -> p dt s", p=P),
            )
            for half in range(2):  # N=512 matmul tiles
                scol = c0 + 512 * half
                for proj in range(3):  # q, k, v
                    src = xs if proj == 0 else xg
                    for dto in range(2):
                        ps = psum_mm.tile([P, 512], FP32, tag="mm512", name="ps_proj")
                        for dt in range(8):
                            nc.tensor.matmul(
                                ps[:],
                                wqkv_s[:, dt * 3 * CW + CW * proj + P * dto : dt * 3 * CW + CW * proj + P * dto + P],
                                src[:, dt * 1024 + 512 * half : dt * 1024 + 512 * half + 512],
                                start=(dt == 0),
                                stop=(dt == 7),
                            )
                        if proj == 0:
                            nc.any.tensor_copy(out=qT_s[:, S * dto + scol : S * dto + scol + 512], in_=ps[:])
                        elif proj == 1:
                            nc.any.tensor_copy(out=kT_s[:, S * dto + scol : S * dto + scol + 512], in_=ps[:])
                        else:
                            # v: evict to bf16, transpose 128x128 chunks into vhat
                            vt = vt_pool.tile([P, 512], BF16, tag="vt", name="vt")
                            nc.any.tensor_copy(out=vt[:], in_=ps[:])
                            for j in range(4):
                                kb = scol // P + j
                                tr = psum_sc.tile([P, P], BF16, tag="score", name="tr")
                                nc.tensor.transpose(tr[:], vt[:, P * j : P * j + P], ident_s[:])
                                for hh in range(2):  # heads 2*dto, 2*dto+1
                                    h = 2 * dto + hh
                                    nc.any.tensor_copy(
                                        out=vhat[:, VROW * kb + 65 * h : VROW * kb + 65 * h + 64],
                                        in_=tr[:, 64 * hh : 64 * hh + 64],
                                    )

    # ---- phase 2+3: qt-major pipeline. Strips of the 4 heads are emitted
    # interleaved (independent chains keep the PE dense -> HAM stays at full
    # clock); each qt's wo matmuls run as soon as its attnT is normalized,
    # overlapping the next qt's strips.
    strips = {h: {} for h in range(HPC)}
    for qt in range(NQT):
        with nc.named_scope(f"attn_q{qt}"):
            for kb in range(4 * qt, 4 * qt + 4):
                for h in range(HPC):
                    ht, hp = h // 2, 64 * (h % 2)
                    w = _strip_width(kb)
                    n1 = min(512, w)
                    n2 = w - n1
                    ps = psum_sc.tile([P, w], FP32, tag="score", name="ps_sc")
                    lhsT = kT_s[hp : hp + 64, S * ht + P * kb : S * ht + P * kb + P]
                    nc.tensor.matmul(
                        ps[:, 0:n1],
                        lhsT,
                        qT_s[hp : hp + 64, S * ht + P * kb : S * ht + P * kb + n1],
                        start=True,
                        stop=True,
                    )
                    if n2:
                        nc.tensor.matmul(
                            ps[:, 512 : 512 + n2],
                            lhsT,
                            qT_s[hp : hp + 64, S * ht + P * kb + 512 : S * ht + P * kb + 512 + n2],
                            start=True,
                            stop=True,
                        )
                    st = strip_pool.tile([P, w], BF16, tag="strip", name="strip")
                    nc.scalar.activation(
                        st[:], ps[:], mybir.ActivationFunctionType.Exp, scale=float(HD) ** -0.5
                    )
                    # zero invalid triangles: diag block (cols 0:128), tail block
                    nc.vector.tensor_mul(out=st[:, 0:P], in0=st[:, 0:P], in1=mask_s[:, 0:P])
                    if n2:
                        nc.vector.tensor_mul(
                            out=st[:, 512 : 512 + n2],
                            in0=st[:, 512 : 512 + n2],
                            in1=mask_s[:, P : P + n2],
                        )
                    strips[h][kb] = st

            for h in range(HPC):
                ht, hp = h // 2, 64 * (h % 2)
                pv = psum_pv.tile([65, 512], FP32, tag="pv", name="ps_pv")
                for qb in range(4 * qt, 4 * qt + 4):
                    for kb2 in range(max(0, qb - 4), qb + 1):
                        nc.tensor.matmul(
                            pv[:, P * (qb - 4 * qt) : P * (qb - 4 * qt) + P],
                            vhat[:, VROW * kb2 + 65 * h : VROW * kb2 + 65 * h + 65],
                            strips[h][kb2][:, P * (qb - kb2) : P * (qb - kb2) + P],
                            start=(kb2 == max(0, qb - 4)),
                            stop=(kb2 == qb),
                        )
                zs = z_pool.tile([1, 512], FP32, tag="zs", name="zs")
                nc.any.tensor_copy(out=zs[:], in_=pv[64:65, :])
                zr = z_pool.tile([1, 512], FP32, tag="zr", name="zr")
                nc.vector.reciprocal_approx_fast(out=zr[:], in_=zs[:])
                zrb = z_pool.tile([64, 512], FP32, tag="zrb", name="zrb")
                nc.gpsimd.partition_broadcast(zrb[:], zr[:])
                nc.vector.tensor_mul(
                    out=attnT_q[qt][hp : hp + 64, 512 * ht : 512 * ht + 512],
                    in0=pv[0:64, :],
                    in1=zrb[:],
                )

        with nc.named_scope(f"wo_q{qt}"):
            for ot in range(8):
                ps = psum_mm.tile([P, 512], FP32, tag="mm512", name="ps_wo")
                for jt in range(2):
                    nc.tensor.matmul(
                        ps[:],
                        woT_s[:, D * jt + P * ot : D * jt + P * ot + P],
                        attnT_q[qt][:, 512 * jt : 512 * jt + 512],
                        start=(jt == 0),
                        stop=(jt == 1),
                    )
                ob = out_pool.tile([P, 512], FP32, tag="wo", name="ob")
                nc.any.tensor_copy(out=ob[:], in_=ps[:])
                nc.sync.dma_start(out=outT[P * ot : P * ot + P, 512 * qt : 512 * qt + 512], in_=ob[:])


_GRAPH_CACHE = {}


def _build():
    if "nc" in _GRAPH_CACHE:
        return _GRAPH_CACHE["nc"]
    nc = bacc.Bacc("TRN2", target_bir_lowering=False, debug=False, num_devices=NCORES)
    xT = nc.dram_tensor("xT", [D, S], BF16, kind="ExternalInput")
    xTg = nc.dram_tensor("xTg", [D, S], BF16, kind="ExternalInput")
    wqkvT = nc.dram_tensor("wqkvT", [D, 3 * CW], BF16, kind="ExternalInput")
    woT = nc.dram_tensor("woT", [CW, D], BF16, kind="ExternalInput")
    mask2 = nc.dram_tensor("mask2", [P, 2 * P], BF16, kind="ExternalInput")
    ident = nc.dram_tensor("ident", [P, P], BF16, kind="ExternalInput")
    outT = nc.dram_tensor("outT", [D, S], FP32, kind="ExternalOutput")
    from contextlib import ExitStack

    with tile.TileContext(nc) as tc, ExitStack() as ctx:
        _emit(ctx, nc, tc, xT, xTg, wqkvT, woT, mask2, ident, outT)
    nc.compile()
    _GRAPH_CACHE["nc"] = nc
    return nc


def _host_masks():
    p = np.arange(P)[:, None]
    c = np.arange(P)[None, :]
    diag = (p <= c).astype(np.float32)   # causal within the diagonal block
    tail = (p > c).astype(np.float32)    # q-k <= 511 within the tail block
    return np.concatenate([diag, tail], axis=1)


def _token_gather(block_tables, slot_mapping):
    """Compose cache scatter (slot_mapping) with block_tables gather into a
    single token index map g[b, t] -> row of x_flat."""
    t = np.arange(S)
    slots = block_tables[:, t // BLOCK].astype(np.int64) * BLOCK + (t % BLOCK)[None, :]
    sm = np.asarray(slot_mapping).astype(np.int64)
    sm_inv = np.empty_like(sm)
    sm_inv[sm] = np.arange(sm.size)
    return sm_inv[slots]  # [B, S]


def make_in_maps(x, wq, wk, wv, wo, block_tables, slot_mapping):
    bf = mybir.dt.np(BF16)
    g = _token_gather(np.asarray(block_tables), np.asarray(slot_mapping))
    x_flat = np.ascontiguousarray(np.asarray(x, dtype=np.float32).reshape(B * S, D))
    mask2 = _host_masks().astype(bf)
    ident = np.eye(P, dtype=np.float32).astype(bf)
    wq, wk, wv, wo = (np.asarray(a, dtype=np.float32) for a in (wq, wk, wv, wo))

    xT_b, xTg_b = [], []
    for bi in range(B):
        xT_b.append(np.ascontiguousarray(x_flat[bi * S : (bi + 1) * S].T.astype(bf)))
        gb = g[bi]
        if np.array_equal(gb, np.arange(bi * S, (bi + 1) * S)):
            xTg_b.append(xT_b[-1])
        else:
            xTg_b.append(np.ascontiguousarray(x_flat[gb].T.astype(bf)))

    in_maps = []
    for c in range(NCORES):
        bi, hg = c // 4, c % 4
        rows = slice(CW * hg, CW * hg + CW)
        wqkvT = np.ascontiguousarray(
            np.concatenate([wq[rows].T, wk[rows].T, wv[rows].T], axis=1).astype(bf)
        )
        woT = np.ascontiguousarray(wo[:, rows].T.astype(bf))
        in_maps.append(
            {
                "xT": xT_b[bi],
                "xTg": xTg_b[bi],
                "wqkvT": wqkvT,
                "woT": woT,
                "mask2": mask2,
                "ident": ident,
            }
        )
    return in_maps


def kernel(x, wq, wk, wv, wo, block_tables, slot_mapping, context_lens, window_size, **run_kwargs):
    assert int(window_size) == WINDOW, f"kernel hardcodes window {WINDOW}"
    assert tuple(np.asarray(x).shape) == (B, S, D)
    nc = _build()
    in_maps = make_in_maps(x, wq, wk, wv, wo, block_tables, slot_mapping)
    res = run_bass_kernel_spmd(nc, in_maps, core_ids=list(range(NCORES)), **run_kwargs)
    outs = [r["outT"].astype(np.float32) for r in res.results]
    out = np.stack(
        [sum(outs[4 * bi : 4 * bi + 4]).T for bi in range(B)]
    ).reshape(B, S, D)
    # context_lens == S for these inputs (full visibility); asserted cheaply
    assert np.all(np.asarray(context_lens) == S)
    if run_kwargs:
        kernel.last_result = res
    return out
